# revision 9
# baseline (speedup 1.0000x reference)
"""Trainium2 Bass kernel for a dense pre-LN transformer block.

Problem: B=2, T=2048, C=1024, H=16 heads (d=64), FFN 4x, causal attention.

Parallelization over 8 NeuronCores (single SPMD program, one launch):
  - LN1 phase: token-parallel. Core c owns tokens of (batch c//4, quarter
    c%4): LN1 + transpose for its 512 tokens only, then an 8-core
    AllGather distributes hT (bf16) to every core.
  - Attention phase: head-tensor-parallel. Core c computes heads {2c, 2c+1}
    for BOTH batches: Q/K/V projections from the gathered hT, causal-block
    attention with unnormalized softmax (denominator via an appended
    ones-column in V).
  - One 8-core AllToAll per head redistributes attn^T from head-split to
    (batch, token)-split; the first A2A overlaps the second head's attention.
  - Post-A2A phase: core c owns its 512 tokens again: receiver-side softmax
    normalization (single fast-reciprocal), output projection + residual,
    LN2, FFN, residual; returns its 512x1024 slice of the output.

All matmuls run with bf16 operands (full PE rate); LN statistics via
bn_stats/bn_aggr in fp32 with one batched Sqrt per LN (avoids ACT
table-set thrash against the softmax Exp).
"""

import numpy as np
import ml_dtypes

B, T, C = 2, 2048, 1024
H, D = 16, 64
FF = 4 * C
EPS = 1e-5
NCORES = 8
TSL = 512  # tokens owned per core (LN1 + post-A2A phases)
BT = B * T  # 4096

_CACHE = {}


# --------------------------------------------------------------------------
# device program
# --------------------------------------------------------------------------
def _build_program():
    import concourse.bass as bass
    import concourse.mybir as mybir
    import concourse.tile as tile
    from concourse import bacc

    dt = mybir.dt
    f32, f32r, bf16 = dt.float32, dt.float32r, dt.bfloat16

    nc = bacc.Bacc("TRN2", target_bir_lowering=False, debug=False,
                   num_devices=NCORES)

    # ---- I/O ----
    x_own = nc.dram_tensor("x_own", [TSL, C], f32, kind="ExternalInput")
    wq2 = nc.dram_tensor("wq2", [C, 128], bf16, kind="ExternalInput")
    wk2 = nc.dram_tensor("wk2", [C, 128], bf16, kind="ExternalInput")
    wv_aug = nc.dram_tensor("wv_aug", [C, 130], bf16, kind="ExternalInput")
    onespat = nc.dram_tensor("onespat", [1, 130], bf16, kind="ExternalInput")
    ones_b = nc.dram_tensor("ones_b", [1, 128], bf16, kind="ExternalInput")
    masks = nc.dram_tensor("masks", [4, 128, 512], bf16, kind="ExternalInput")
    wproj = nc.dram_tensor("wproj", [C, C], bf16, kind="ExternalInput")
    bproj = nc.dram_tensor("bproj", [1, C], bf16, kind="ExternalInput")
    w1 = nc.dram_tensor("w1", [C, FF], bf16, kind="ExternalInput")
    w2 = nc.dram_tensor("w2", [FF, C], bf16, kind="ExternalInput")
    b1t = nc.dram_tensor("b1t", [128, FF // 128], f32, kind="ExternalInput")
    b2row = nc.dram_tensor("b2row", [1, C], bf16, kind="ExternalInput")
    g1t = nc.dram_tensor("g1t", [128, 8], f32, kind="ExternalInput")
    be1t = nc.dram_tensor("be1t", [128, 8], f32, kind="ExternalInput")
    g2t = nc.dram_tensor("g2t", [128, 8], f32, kind="ExternalInput")
    be2t = nc.dram_tensor("be2t", [128, 8], f32, kind="ExternalInput")
    ident = nc.dram_tensor("ident", [128, 128], f32, kind="ExternalInput")
    identb = nc.dram_tensor("identb", [128, 128], bf16, kind="ExternalInput")
    out = nc.dram_tensor("out", [TSL, C], f32, kind="ExternalOutput")

    with tile.TileContext(nc, num_cores=NCORES) as tc:
        _body(nc, tc, tile, mybir, bass, locals())
    nc.compile()
    return nc


def _body(nc, tc, tile, mybir, bass, io):
    dt = mybir.dt
    f32, f32r, bf16 = dt.float32, dt.float32r, dt.bfloat16
    AF = mybir.ActivationFunctionType
    OP = mybir.AluOpType

    x_own = io["x_own"]
    wq2, wk2, wv_aug = io["wq2"], io["wk2"], io["wv_aug"]
    onespat, ones_b = io["onespat"], io["ones_b"]
    masks, wproj, bproj = io["masks"], io["wproj"], io["bproj"]
    w1, w2, b1t = io["w1"], io["w2"], io["b1t"]
    b2row = io["b2row"]
    g1t, be1t, g2t, be2t = io["g1t"], io["be1t"], io["g2t"], io["be2t"]
    ident, identb, out = io["ident"], io["identb"], io["out"]

    # ---- persistent pools ----
    consts = tc.alloc_tile_pool(name="consts", bufs=1)
    persA = tc.alloc_tile_pool(name="persA", bufs=1)  # attention lifetime
    dram = tc.alloc_tile_pool(name="dram", bufs=1, space="DRAM")

    id_sb = consts.tile([128, 128], f32, name="id_sb")
    nc.sync.dma_start(out=id_sb[:], in_=ident[:])
    idb_sb = consts.tile([128, 128], bf16, name="idb_sb")
    nc.sync.dma_start(out=idb_sb[:], in_=identb[:])
    wq_sb = consts.tile([128, 8, 128], bf16, name="wq_sb")
    nc.sync.dma_start(out=wq_sb[:], in_=wq2[:].rearrange("(cc p) d -> p cc d", p=128))
    wk_sb = consts.tile([128, 8, 128], bf16, name="wk_sb")
    nc.sync.dma_start(out=wk_sb[:], in_=wk2[:].rearrange("(cc p) d -> p cc d", p=128))
    wv_sb = consts.tile([128, 8, 130], bf16, name="wv_sb")
    nc.sync.dma_start(out=wv_sb[:], in_=wv_aug[:].rearrange("(cc p) d -> p cc d", p=128))
    onespat_sb = consts.tile([1, 130], bf16, name="onespat_sb")
    nc.sync.dma_start(out=onespat_sb[:], in_=onespat[:])
    ones_b_sb = consts.tile([1, 128], bf16, name="ones_b_sb")
    nc.sync.dma_start(out=ones_b_sb[:], in_=ones_b[:])
    g1_sb = consts.tile([128, 8], f32, name="g1_sb")
    nc.sync.dma_start(out=g1_sb[:], in_=g1t[:])
    be1_sb = consts.tile([128, 8], f32, name="be1_sb")
    nc.sync.dma_start(out=be1_sb[:], in_=be1t[:])
    g2_sb = consts.tile([128, 8], f32, name="g2_sb")
    nc.sync.dma_start(out=g2_sb[:], in_=g2t[:])
    be2_sb = consts.tile([128, 8], f32, name="be2_sb")
    nc.sync.dma_start(out=be2_sb[:], in_=be2t[:])
    b1_sb = consts.tile([128, FF // 128], f32, name="b1_sb")
    nc.sync.dma_start(out=b1_sb[:], in_=b1t[:])
    b2r_sb = consts.tile([1, C], bf16, name="b2r_sb")
    nc.sync.dma_start(out=b2r_sb[:], in_=b2row[:])
    bproj_sb = consts.tile([1, C], bf16, name="bproj_sb")
    nc.sync.dma_start(out=bproj_sb[:], in_=bproj[:])
    mask_sb = consts.tile([128, 4, 512], bf16, name="mask_sb")
    nc.sync.dma_start(out=mask_sb[:], in_=masks[:].rearrange("i p t -> p i t"))
    eps_sb = consts.tile([128, 1], f32, name="eps_sb")
    nc.vector.memset(eps_sb[:], EPS)

    # own-token residual input, f32: [128, blk, C]
    xo = consts.tile([128, 4, C], f32, name="xo")
    nc.sync.dma_start(out=xo[:], in_=x_own[:].rearrange("(tq p) e -> p tq e", p=128))

    # attention-persistent tensors, per batch (bf16 pipeline)
    qT = [persA.tile([128, T], bf16, name=f"qTb{b}") for b in range(2)]
    kT = [persA.tile([128, T], bf16, name=f"kTb{b}") for b in range(2)]
    vaug = [persA.tile([128, 16 * 130], bf16, name=f"vaugb{b}") for b in range(2)]
    aT_h = [[persA.tile([64, T], bf16, name=f"aTb{b}h{h}") for h in range(2)]
            for b in range(2)]

    ag_in = dram.tile([128, 8, 512], bf16, name="ag_in")
    ag_out = dram.tile([8, 128, 8, 512], bf16, name="ag_out",
                       addr_space="Shared")
    a2a_in = [dram.tile([8, 65, 512], bf16, name=f"a2a_in{hh}")
              for hh in range(2)]
    a2a_out = [dram.tile([8, 65, 512], bf16, name=f"a2a_out{hh}")
               for hh in range(2)]

    # ======================================================================
    # Phase A: token-sharded LN1 + transpose; AllGather hT; head-sharded QKV
    # ======================================================================
    hT_own = persA.tile([128, 8, 512], bf16, name="hT_own")
    with tc.tile_pool(name="lnA", bufs=1) as lnA, \
         tc.tile_pool(name="psA0", bufs=1, space="PSUM") as psA0, \
         nc.named_scope("ln1"):
        mvall = lnA.tile([128, 4, 2], f32, name="mvall")
        for blk in range(4):
            st = lnA.tile([128, 2, 6], f32, tag="st", bufs=2,
                          name=f"st_{blk}")
            nc.vector.bn_stats(out=st[:, 0, :], in_=xo[:, blk, 0:512])
            nc.vector.bn_stats(out=st[:, 1, :], in_=xo[:, blk, 512:1024])
            nc.vector.bn_aggr(out=mvall[:, blk, :], in_=st[:])
        # batched rsqrt: one Sqrt call for all 4 blocks (no table thrash)
        sd = lnA.tile([128, 4], f32, name="sd")
        nc.scalar.activation(out=sd[:], in_=mvall[:, :, 1], func=AF.Sqrt,
                             bias=eps_sb[:])
        rsall = lnA.tile([128, 4], f32, name="rsall")
        nc.vector.reciprocal(out=rsall[:], in_=sd[:])
        hsubs = []
        for blk in range(4):
            h = lnA.tile([128, C], bf16, tag="h", bufs=4, name=f"h_{blk}")
            nc.vector.tensor_scalar(out=h[:], in0=xo[:, blk, :],
                                    scalar1=mvall[:, blk, 0:1],
                                    scalar2=rsall[:, blk:blk + 1],
                                    op0=OP.subtract, op1=OP.mult)
            hsubs.append(h)
        for cc in range(8):
            pth = psA0.tile([128, 512], bf16, tag="pth", bufs=2,
                            name=f"pth_{cc}")
            for blk in range(4):
                nc.tensor.transpose(
                    pth[:, blk * 128:(blk + 1) * 128],
                    hsubs[blk][:, cc * 128:(cc + 1) * 128], idb_sb[:])
            nc.vector.tensor_scalar(out=hT_own[:, cc, :], in0=pth[:],
                                    scalar1=g1_sb[:, cc:cc + 1],
                                    scalar2=be1_sb[:, cc:cc + 1],
                                    op0=OP.mult, op1=OP.add)
        nc.sync.dma_start(out=ag_in[:], in_=hT_own[:])
    nc.gpsimd.collective_compute(
        "AllGather", mybir.AluOpType.bypass,
        replica_groups=[list(range(NCORES))],
        ins=[ag_in[:].opt()], outs=[ag_out[:].opt()])

    # hT for all tokens: hTg[r] = tokens of (batch r//4, quarter r%4)
    hTg_pool = tc.alloc_tile_pool(name="hTg", bufs=1)
    hTg = [hTg_pool.tile([128, 8, 512], bf16, name=f"hTg_{r}")
           for r in range(8)]
    for r in range(8):
        nc.sync.dma_start(out=hTg[r][:], in_=ag_out[r])

    with tc.tile_pool(name="qkvp", bufs=1) as qkvp, \
         tc.tile_pool(name="psQ", bufs=1, space="PSUM") as psQ:
        for b in range(2):
            with nc.named_scope(f"qkv_b{b}"):
                for q4 in range(4):
                    src = hTg[b * 4 + q4]
                    col = q4 * 512
                    for w_sb, dst in ((wq_sb, qT[b]), (wk_sb, kT[b])):
                        pqk = psQ.tile([128, 512], f32, tag="pqv", bufs=2,
                                       name=f"pqk_{b}_{q4}_{dst.name}")
                        for cc in range(8):
                            nc.tensor.matmul(pqk[:], w_sb[:, cc, :],
                                             src[:, cc, :],
                                             start=(cc == 0), stop=(cc == 7))
                        nc.vector.tensor_copy(out=dst[:, col:col + 512], in_=pqk[:])
                    # v (+ ones column), bf16
                    for sc in range(4):
                        sb = q4 * 4 + sc
                        pv = psQ.tile([128, 512], f32, tag="pqv", bufs=2,
                                      name=f"pv_{b}_{sb}")
                        for cc in range(8):
                            nc.tensor.matmul(
                                pv[:, 0:130], src[:, cc, sc * 128:(sc + 1) * 128],
                                wv_sb[:, cc, :], start=(cc == 0), stop=False)
                        nc.tensor.matmul(pv[:, 0:130], ones_b_sb[:], onespat_sb[:],
                                         start=False, stop=True)
                        nc.vector.tensor_copy(out=vaug[b][:, sb * 130:(sb + 1) * 130],
                                              in_=pv[:, 0:130])
    hTg_pool.release()

    wp_sb = consts.tile([128, 8, C], bf16, name="wp_sb")
    nc.sync.dma_start(out=wp_sb[:],
                      in_=wproj[:].rearrange("(dc p) e -> p dc e", p=128))

    # ======================================================================
    # Phase B: causal attention, head-major; head-0 A2A hides under head-1
    # ======================================================================
    with tc.tile_pool(name="attnp", bufs=1) as lnA, \
         tc.tile_pool(name="psAB", bufs=1, space="PSUM") as psA:
        dens = [None, None]
        for b in range(2):
            dens[b] = lnA.tile([65, 2, T], bf16, tag="den", bufs=2,
                               name=f"den_{b}")  # row 64: softmax denominators
        for h in range(2):
            hp = 64 * h
            for b in range(2):
                den = dens[b]
                with nc.named_scope(f"attn_b{b}h{h}"):
                    for half in range(2):  # query chunks (2*half, 2*half+1)
                        qc0 = half * 1024
                        pat = [psA.tile([65, 512], f32, tag="pat", bufs=2,
                                        name=f"pat_{b}_{h}_{half}_{i}")
                               for i in range(2)]
                        nsb = 8 * half + 8
                        for sb in range(nsb):
                            # active query chunks of this half (causal)
                            act0 = 0 if sb < 8 * half + 4 else 1
                            dtc = sb // 4 - 2 * half  # diag chunk idx in half
                            ps = psA.tile([128, 1024], f32, tag="ps", bufs=2,
                                          name=f"ps_{b}_{h}_{half}_{sb}")
                            for i in range(act0, 2):
                                nc.tensor.matmul(
                                    ps[:, i * 512:(i + 1) * 512],
                                    kT[b][hp:hp + 64, sb * 128:sb * 128 + 128],
                                    qT[b][hp:hp + 64,
                                          qc0 + i * 512:qc0 + (i + 1) * 512],
                                    start=True, stop=True)
                            pt = lnA.tile([128, 1024], bf16, tag="pt", bufs=4,
                                          name=f"pt_{b}_{h}_{half}_{sb}")
                            nc.scalar.activation(out=pt[:, act0 * 512:1024],
                                                 in_=ps[:, act0 * 512:1024],
                                                 func=AF.Exp, scale=0.125)
                            if dtc >= act0:
                                nc.vector.tensor_mul(
                                    pt[:, dtc * 512:(dtc + 1) * 512],
                                    pt[:, dtc * 512:(dtc + 1) * 512],
                                    mask_sb[:, sb % 4, :])
                            vs = sb * 130 + 65 * h
                            for i in range(act0, 2):
                                last = 8 * half + 3 if i == 0 else nsb - 1
                                nc.tensor.matmul(
                                    pat[i][:], vaug[b][:, vs:vs + 65],
                                    pt[:, i * 512:(i + 1) * 512],
                                    start=(sb == 0), stop=(sb == last))
                        for i in range(2):
                            tc4 = 2 * half + i
                            qcol = tc4 * 512
                            nc.vector.tensor_copy(
                                out=aT_h[b][h][:, qcol:qcol + 512],
                                in_=pat[i][0:64, :])
                            nc.vector.tensor_copy(
                                out=den[64:65, h, qcol:qcol + 512],
                                in_=pat[i][64:65, :])
            # shard DMAs + collective for this head (first one overlaps the
            # second head's attention)
            for j in range(8):
                bj, tq = j // 4, j % 4
                scol = tq * 512
                nc.sync.dma_start(out=a2a_in[h][j, 0:64, :],
                                  in_=aT_h[bj][h][:, scol:scol + 512])
                nc.sync.dma_start(out=a2a_in[h][j, 64:65, :],
                                  in_=dens[bj][64:65, h, scol:scol + 512])
            nc.gpsimd.collective_compute(
                "AllToAll", mybir.AluOpType.bypass,
                replica_groups=[list(range(NCORES))],
                ins=[a2a_in[h][:].opt()], outs=[a2a_out[h][:].opt()])
    persA.release()

    # ======================================================================
    # Phases D+E: projection + residual + LN2 + FFN + output
    # ======================================================================
    persD = tc.alloc_tile_pool(name="persD", bufs=1)
    x2 = persD.tile([128, 4, C], f32, name="x2")
    h2T = persD.tile([128, 8, 512], bf16, name="h2T")
    ff1T = persD.tile([128, 32, 512], bf16, name="ff1T")
    with tc.tile_pool(name="prDE", bufs=1) as prD:
        aT_own = prD.tile([128, 8, 512], bf16, tag="aT_own", name="aT_own")
        for r in range(8):
            nc.sync.dma_start(out=aT_own[0:64, r, :],
                              in_=a2a_out[0][r, 0:64, :])
            nc.sync.dma_start(out=aT_own[64:128, r, :],
                              in_=a2a_out[1][r, 0:64, :])
        # receiver-side softmax normalization: one gather of the 16 den rows,
        # one fast reciprocal, then broadcast + multiply per r-block
        dgb = prD.tile([16, 512], bf16, tag="dgb", name="dgb")
        for hh in range(2):
            for r in range(8):
                nc.sync.dma_start(out=dgb[hh * 8 + r:hh * 8 + r + 1, :],
                                  in_=a2a_out[hh][r, 64:65, :])
        dg = prD.tile([16, 512], f32, tag="dg", name="dg")
        nc.vector.tensor_copy(out=dg[:], in_=dgb[:])
        dr = prD.tile([16, 512], f32, tag="dr", name="dr")
        nc.vector.reciprocal_approx_fast(out=dr[:], in_=dg[:])
        drb = prD.tile([16, 512], bf16, tag="drb", name="drb")
        nc.vector.tensor_copy(out=drb[:], in_=dr[:])
        dr_dram = dram.tile([16, 512], bf16, name="dr_dram")
        nc.sync.dma_start(out=dr_dram[:], in_=drb[:])
        for r in range(8):
            rb = prD.tile([128, 512], bf16, tag="rb", bufs=3, name=f"rb_{r}")
            nc.sync.dma_start(out=rb[0:64, :],
                              in_=dr_dram[r:r + 1, :].to_broadcast([64, 512]))
            nc.sync.dma_start(out=rb[64:128, :],
                              in_=dr_dram[8 + r:8 + r + 1, :].to_broadcast([64, 512]))
            nc.vector.tensor_mul(aT_own[:, r, :], aT_own[:, r, :], rb[:])
        with tc.tile_pool(name="psD", bufs=1, space="PSUM") as psD, \
             nc.named_scope("proj_ln2"):
            mv2all = prD.tile([128, 4, 2], f32, tag="mv2all", name="mv2all")
            for tq in range(4):
                for eh in range(2):
                    pp = psD.tile([128, 512], f32, tag="pp", bufs=2,
                                  name=f"pp_{tq}_{eh}")
                    for dc in range(8):
                        nc.tensor.matmul(
                            pp[:], aT_own[:, dc, tq * 128:(tq + 1) * 128],
                            wp_sb[:, dc, eh * 512:eh * 512 + 512],
                            start=(dc == 0), stop=False)
                    nc.tensor.matmul(pp[:], ones_b_sb[:],
                                     bproj_sb[0:1, eh * 512:eh * 512 + 512],
                                     start=False, stop=True)
                    nc.vector.tensor_add(x2[:, tq, eh * 512:eh * 512 + 512],
                                         pp[:], xo[:, tq, eh * 512:eh * 512 + 512])
                st2 = prD.tile([128, 2, 6], f32, tag="st2", bufs=2,
                               name=f"st2_{tq}")
                nc.vector.bn_stats(out=st2[:, 0, :], in_=x2[:, tq, 0:512])
                nc.vector.bn_stats(out=st2[:, 1, :], in_=x2[:, tq, 512:1024])
                nc.vector.bn_aggr(out=mv2all[:, tq, :], in_=st2[:])
            sd2 = prD.tile([128, 4], f32, tag="sd2", name="sd2")
            nc.scalar.activation(out=sd2[:], in_=mv2all[:, :, 1], func=AF.Sqrt,
                                 bias=eps_sb[:])
            rs2all = prD.tile([128, 4], f32, tag="rs2all", name="rs2all")
            nc.vector.reciprocal(out=rs2all[:], in_=sd2[:])
            h2subs = []
            for tq in range(4):
                h2 = prD.tile([128, C], f32, tag="h2", bufs=4, name=f"h2_{tq}")
                nc.vector.tensor_scalar(out=h2[:], in0=x2[:, tq, :],
                                        scalar1=mv2all[:, tq, 0:1],
                                        scalar2=rs2all[:, tq:tq + 1],
                                        op0=OP.subtract, op1=OP.mult)
                h2subs.append(h2)
            for cc in range(8):
                pt2 = psD.tile([128, 512], f32, tag="pt2", bufs=2,
                               name=f"pt2_{cc}")
                for tq in range(4):
                    nc.tensor.transpose(pt2[:, tq * 128:(tq + 1) * 128],
                                        h2subs[tq][:, cc * 128:(cc + 1) * 128],
                                        id_sb[:])
                nc.vector.tensor_scalar(out=h2T[:, cc, :], in0=pt2[:],
                                        scalar1=g2_sb[:, cc:cc + 1],
                                        scalar2=be2_sb[:, cc:cc + 1],
                                        op0=OP.mult, op1=OP.add)
        w1r = w1[:].rearrange("(cc p) m -> p cc m", p=128)
        with tc.tile_pool(name="ps1", bufs=1, space="PSUM") as ps1, \
             nc.named_scope("ffn1"):
            for w in range(16):  # m-windows of 256
                w1w = prD.tile([128, 8, 256], bf16, tag="w1w", bufs=3,
                               name=f"w1w_{w}")
                nc.sync.dma_start(out=w1w[:], in_=w1r[:, :, w * 256:(w + 1) * 256])
                for m2 in range(2):
                    m = w * 2 + m2  # m-chunk of 128
                    pf = ps1.tile([128, 512], f32, tag="pf", bufs=3,
                                  name=f"pf_{m}")
                    for cc in range(8):
                        nc.tensor.matmul(
                            pf[:], w1w[:, cc, m2 * 128:(m2 + 1) * 128],
                            h2T[:, cc, :], start=(cc == 0), stop=(cc == 7))
                    nc.scalar.activation(out=ff1T[:, m, :], in_=pf[:],
                                         func=AF.Relu, bias=b1_sb[:, m:m + 1])
        # FFN2 with ff1T stationary (reused across both 512-col halves of
        # w2's row) / w2 rows moving: output lands directly as ff[t, e].
        with tc.tile_pool(name="ps2", bufs=1, space="PSUM") as ps2p, \
             nc.named_scope("ffn2"):
            pso = [ps2p.tile([128, C], f32, tag="pso", bufs=4, name=f"pso_{tq}")
                   for tq in range(4)]
            for mc in range(32):
                w2t = prD.tile([128, C], bf16, tag="w2t", bufs=6,
                               name=f"w2t_{mc}")
                nc.sync.dma_start(out=w2t[:], in_=w2[mc * 128:(mc + 1) * 128, :])
                for tq in range(4):
                    for eh in range(2):
                        nc.tensor.matmul(pso[tq][:, eh * 512:(eh + 1) * 512],
                                         ff1T[:, mc, tq * 128:(tq + 1) * 128],
                                         w2t[:, eh * 512:(eh + 1) * 512],
                                         start=(mc == 0), stop=False)
            for tq in range(4):
                for eh in range(2):
                    nc.tensor.matmul(pso[tq][:, eh * 512:(eh + 1) * 512],
                                     ones_b_sb[:], b2r_sb[0:1, eh * 512:(eh + 1) * 512],
                                     start=False, stop=True)
                ot = prD.tile([128, C], f32, tag="ot", bufs=2, name=f"ot_{tq}")
                nc.vector.tensor_add(ot[:], pso[tq][:], x2[:, tq, :])
                nc.sync.dma_start(out=out[tq * 128:(tq + 1) * 128, :], in_=ot[:])
    persD.release()
    consts.release()
    dram.release()


# --------------------------------------------------------------------------
# host driver
# --------------------------------------------------------------------------
def _make_in_maps(inputs):
    x = np.ascontiguousarray(np.asarray(inputs["x"], np.float32))
    wq = np.asarray(inputs["wq"], np.float32)
    wk = np.asarray(inputs["wk"], np.float32)
    wv = np.asarray(inputs["wv"], np.float32)
    w_proj = np.ascontiguousarray(np.asarray(inputs["w_proj"], np.float32))
    b_proj = np.asarray(inputs["b_proj"], np.float32)
    w1 = np.ascontiguousarray(np.asarray(inputs["w1"], np.float32))
    b1 = np.asarray(inputs["b1"], np.float32)
    w2 = np.ascontiguousarray(np.asarray(inputs["w2"], np.float32))
    b2 = np.asarray(inputs["b2"], np.float32)
    g1 = np.asarray(inputs["g1"], np.float32)
    be1 = np.asarray(inputs["be1"], np.float32)
    g2 = np.asarray(inputs["g2"], np.float32)
    be2 = np.asarray(inputs["be2"], np.float32)

    xf = x.reshape(BT, C)
    i_mask = np.zeros((4, 128, 512), np.float32)
    s_idx = np.arange(128)[:, None]
    t_idx = np.arange(512)[None, :]
    for i in range(4):
        i_mask[i] = (s_idx + 128 * i <= t_idx).astype(np.float32)
    onespat = np.zeros((1, 130), np.float32)
    onespat[0, 64] = 1.0
    onespat[0, 129] = 1.0

    common = dict(
        masks=i_mask.astype(ml_dtypes.bfloat16),
        onespat=onespat.astype(ml_dtypes.bfloat16),
        ones_b=np.ones((1, 128), ml_dtypes.bfloat16),
        wproj=w_proj.astype(ml_dtypes.bfloat16),
        bproj=np.ascontiguousarray(b_proj[None, :]).astype(ml_dtypes.bfloat16),
        w1=w1.astype(ml_dtypes.bfloat16), w2=w2.astype(ml_dtypes.bfloat16),
        b1t=np.ascontiguousarray(b1.reshape(FF // 128, 128).T),
        b2row=np.ascontiguousarray(b2[None, :]).astype(ml_dtypes.bfloat16),
        g1t=np.ascontiguousarray(g1.reshape(8, 128).T),
        be1t=np.ascontiguousarray(be1.reshape(8, 128).T),
        g2t=np.ascontiguousarray(g2.reshape(8, 128).T),
        be2t=np.ascontiguousarray(be2.reshape(8, 128).T),
        ident=np.eye(128, dtype=np.float32),
        identb=np.eye(128).astype(ml_dtypes.bfloat16),
    )
    in_maps = []
    for c in range(NCORES):
        b, hg = c // 4, c % 4
        wva = np.zeros((C, 130), np.float32)
        wva[:, 0:64] = wv[2 * c]
        wva[:, 65:129] = wv[2 * c + 1]
        m = dict(common)
        m["x_own"] = np.ascontiguousarray(
            xf[b * T + hg * TSL: b * T + (hg + 1) * TSL])
        m["wq2"] = np.ascontiguousarray(
            np.concatenate([wq[2 * c], wq[2 * c + 1]], axis=1)).astype(
                ml_dtypes.bfloat16)
        m["wk2"] = np.ascontiguousarray(
            np.concatenate([wk[2 * c], wk[2 * c + 1]], axis=1)).astype(
                ml_dtypes.bfloat16)
        m["wv_aug"] = wva.astype(ml_dtypes.bfloat16)
        in_maps.append(m)
    return in_maps


LAST_RESULTS = None


def kernel(trace=False, **inputs):
    global LAST_RESULTS
    from concourse import bass_utils

    if "nc" not in _CACHE:
        _CACHE["nc"] = _build_program()
    nc = _CACHE["nc"]
    in_maps = _make_in_maps(inputs)
    res = bass_utils.run_bass_kernel_spmd(
        nc, in_maps, core_ids=list(range(NCORES)), trace=trace)
    LAST_RESULTS = res
    out = np.zeros((B, T, C), np.float32)
    for c in range(NCORES):
        b, hg = c // 4, c % 4
        out[b, hg * TSL:(hg + 1) * TSL, :] = res.results[c]["out"]
    return out


# revision 22
# speedup vs baseline: 1.2124x; 1.2124x over previous
"""Trainium2 Bass kernel for a dense pre-LN transformer block.

Problem: B=2, T=2048, C=1024, H=16 heads (d=64), FFN 4x, causal attention.

Parallelization over 8 NeuronCores (single SPMD program, one launch):
  - Attention phase: head-tensor-parallel. Core c computes heads {2c, 2c+1}
    for BOTH batches: LN1 (replicated, engine-balanced across DVE+GpSimd),
    Q/K/V projections, causal-block attention with unnormalized softmax
    (denominator via an appended ones-column in V).
  - Attention runs query-half-major; after each half an 8-core AllToAll
    redistributes that half's attn^T (both heads + denominators) from
    head-split to (batch, token)-split. The first A2A overlaps the second
    half's attention; the second A2A overlaps the first half's
    proj/LN2/FFN1 work.
  - Core c owns tokens [256*(c%4), +256) and [1024+256*(c%4), +256) of
    batch c//4 in the post-A2A phase: softmax normalization (single fast
    reciprocal), output projection + residual, LN2, FFN, residual.

All matmuls run with bf16 operands; LN statistics via bn_stats/bn_aggr in
fp32 with one batched Sqrt per LN block-group (avoids ACT table-set thrash
against the softmax Exp).
"""

import numpy as np
import ml_dtypes

B, T, C = 2, 2048, 1024
H, D = 16, 64
FF = 4 * C
EPS = 1e-5
NCORES = 8
TSL = 512  # tokens owned per core in the post-A2A phase
BT = B * T  # 4096

_CACHE = {}


# --------------------------------------------------------------------------
# device program
# --------------------------------------------------------------------------
def _build_program():
    import concourse.bass as bass
    import concourse.mybir as mybir
    import concourse.tile as tile
    from concourse import bacc

    dt = mybir.dt
    f32, bf16 = dt.float32, dt.bfloat16

    nc = bacc.Bacc("TRN2", target_bir_lowering=False, debug=False,
                   num_devices=NCORES)

    # ---- I/O ----
    x_bf = nc.dram_tensor("x_bf", [BT, C], bf16, kind="ExternalInput")
    x_own = nc.dram_tensor("x_own", [TSL, C], f32, kind="ExternalInput")
    wq2 = nc.dram_tensor("wq2", [C, 128], bf16, kind="ExternalInput")
    wk2 = nc.dram_tensor("wk2", [C, 128], bf16, kind="ExternalInput")
    wv_aug = nc.dram_tensor("wv_aug", [C, 130], bf16, kind="ExternalInput")
    onespat = nc.dram_tensor("onespat", [1, 130], bf16, kind="ExternalInput")
    ones_b = nc.dram_tensor("ones_b", [1, 128], bf16, kind="ExternalInput")
    ones512 = nc.dram_tensor("ones512", [1, 512], bf16, kind="ExternalInput")
    masks = nc.dram_tensor("masks", [4, 128, 512], bf16, kind="ExternalInput")
    wproj = nc.dram_tensor("wproj", [C, C], bf16, kind="ExternalInput")
    bproj = nc.dram_tensor("bproj", [1, C], bf16, kind="ExternalInput")
    w1blk = nc.dram_tensor("w1blk", [16, 128, 8, 256], bf16, kind="ExternalInput")
    w2 = nc.dram_tensor("w2", [FF, C], bf16, kind="ExternalInput")
    b1t = nc.dram_tensor("b1t", [128, FF // 128], f32, kind="ExternalInput")
    b2row = nc.dram_tensor("b2row", [1, C], bf16, kind="ExternalInput")
    bq_row = nc.dram_tensor("bq_row", [1, 128], bf16, kind="ExternalInput")
    bk_row = nc.dram_tensor("bk_row", [1, 128], bf16, kind="ExternalInput")
    ident = nc.dram_tensor("ident", [128, 128], f32, kind="ExternalInput")
    identb = nc.dram_tensor("identb", [128, 128], bf16, kind="ExternalInput")
    out = nc.dram_tensor("out", [TSL, C], f32, kind="ExternalOutput")

    with tile.TileContext(nc, num_cores=NCORES) as tc:
        _body(nc, tc, tile, mybir, bass, locals())
    nc.compile()
    return nc


def _body(nc, tc, tile, mybir, bass, io):
    dt = mybir.dt
    f32, bf16 = dt.float32, dt.bfloat16
    AF = mybir.ActivationFunctionType
    OP = mybir.AluOpType

    x_bf, x_own = io["x_bf"], io["x_own"]
    wq2, wk2, wv_aug = io["wq2"], io["wk2"], io["wv_aug"]
    onespat, ones_b = io["onespat"], io["ones_b"]
    masks, wproj, bproj = io["masks"], io["wproj"], io["bproj"]
    w1blk, w2, b1t = io["w1blk"], io["w2"], io["b1t"]
    b2row = io["b2row"]
    bq_row, bk_row = io["bq_row"], io["bk_row"]
    ident, identb, out = io["ident"], io["identb"], io["out"]

    # ---- persistent pools ----
    consts = tc.alloc_tile_pool(name="consts", bufs=1)
    persA = tc.alloc_tile_pool(name="persA", bufs=1)  # attention lifetime
    dram = tc.alloc_tile_pool(name="dram", bufs=1, space="DRAM")

    id_sb = consts.tile([128, 128], f32, name="id_sb")
    nc.sync.dma_start(out=id_sb[:], in_=ident[:])
    idb_sb = consts.tile([128, 128], bf16, name="idb_sb")
    nc.sync.dma_start(out=idb_sb[:], in_=identb[:])
    wq_sb = consts.tile([128, 8, 128], bf16, name="wq_sb")
    nc.sync.dma_start(out=wq_sb[:], in_=wq2[:].rearrange("(cc p) d -> p cc d", p=128))
    wk_sb = consts.tile([128, 8, 128], bf16, name="wk_sb")
    nc.sync.dma_start(out=wk_sb[:], in_=wk2[:].rearrange("(cc p) d -> p cc d", p=128))
    wv_sb = consts.tile([128, 8, 130], bf16, name="wv_sb")
    nc.sync.dma_start(out=wv_sb[:], in_=wv_aug[:].rearrange("(cc p) d -> p cc d", p=128))
    onespat_sb = consts.tile([1, 130], bf16, name="onespat_sb")
    nc.sync.dma_start(out=onespat_sb[:], in_=onespat[:])
    ones_b_sb = consts.tile([1, 128], bf16, name="ones_b_sb")
    nc.sync.dma_start(out=ones_b_sb[:], in_=ones_b[:])
    ones512 = io["ones512"]
    ones512_sb = consts.tile([1, 512], bf16, name="ones512_sb")
    nc.sync.dma_start(out=ones512_sb[:], in_=ones512[:])
    bq_sb = consts.tile([1, 128], bf16, name="bq_sb")
    nc.sync.dma_start(out=bq_sb[:], in_=bq_row[:])
    bk_sb = consts.tile([1, 128], bf16, name="bk_sb")
    nc.sync.dma_start(out=bk_sb[:], in_=bk_row[:])
    b1_sb = consts.tile([128, FF // 128], f32, name="b1_sb")
    nc.sync.dma_start(out=b1_sb[:], in_=b1t[:])
    b2r_sb = consts.tile([1, C], bf16, name="b2r_sb")
    nc.sync.dma_start(out=b2r_sb[:], in_=b2row[:])
    bproj_sb = consts.tile([1, C], bf16, name="bproj_sb")
    nc.sync.dma_start(out=bproj_sb[:], in_=bproj[:])
    mask_sb = consts.tile([128, 4, 512], bf16, name="mask_sb")
    nc.sync.dma_start(out=mask_sb[:], in_=masks[:].rearrange("i p t -> p i t"))
    eps_sb = consts.tile([128, 1], f32, name="eps_sb")
    nc.vector.memset(eps_sb[:], EPS)
    wp_sb = consts.tile([128, 8, C], bf16, name="wp_sb")
    xo = consts.tile([128, 4, C], f32, name="xo")

    # attention-persistent tensors, per batch (bf16 pipeline)
    qT = [persA.tile([128, T], bf16, name=f"qTb{b}") for b in range(2)]
    kT = [persA.tile([128, T], bf16, name=f"kTb{b}") for b in range(2)]
    vaug = [persA.tile([128, 16 * 130], bf16, name=f"vaugb{b}") for b in range(2)]
    aT_h = [[persA.tile([64, T], bf16, name=f"aTb{b}h{h}") for h in range(2)]
            for b in range(2)]

    a2a_in = [dram.tile([8, 130, 256], bf16, name=f"a2a_in{hf}")
              for hf in range(2)]
    a2a_out = [dram.tile([8, 130, 256], bf16, name=f"a2a_out{hf}")
               for hf in range(2)]
    dr_dram = [dram.tile([16, 256], bf16, name=f"dr_dram{hf}")
               for hf in range(2)]

    # ======================================================================
    # Phase A: LN1 (replicated, DVE+GpSimd balanced) + QKV for own 2 heads
    # ======================================================================
    with tc.tile_pool(name="lnAB", bufs=1) as lnA, \
         tc.tile_pool(name="psAB0", bufs=1, space="PSUM") as psA0:
        # interleave batches so both batches' first token-halves finish
        # early: attention(half 0) can then start while qkv continues
        for b, tch in [(0, 0), (0, 1), (1, 0), (1, 1),
                       (0, 2), (0, 3), (1, 2), (1, 3)]:
            with nc.named_scope(f"qkv_b{b}t{tch}"):
                if True:
                    hsubs = []
                    mvt = lnA.tile([128, 4, 2], f32, tag="mvt", bufs=2,
                                   name=f"mvt_{b}_{tch}")
                    for sub in range(4):
                        row0 = b * T + tch * 512 + sub * 128
                        xt = lnA.tile([128, C], bf16, tag="xt", bufs=6,
                                      name=f"xt_{b}_{tch}_{sub}")
                        nc.sync.dma_start(out=xt[:], in_=x_bf[row0:row0 + 128, :])
                        st = lnA.tile([128, 2, 6], f32, tag="st", bufs=2,
                                      name=f"st_{b}_{tch}_{sub}")
                        nc.vector.bn_stats(out=st[:, 0, :], in_=xt[:, 0:512])
                        nc.vector.bn_stats(out=st[:, 1, :], in_=xt[:, 512:1024])
                        nc.vector.bn_aggr(out=mvt[:, sub, :], in_=st[:])
                        hsubs.append(xt)
                    # one batched sqrt per 512-token chunk (table friendly)
                    sd = lnA.tile([128, 4], f32, tag="sd", bufs=2,
                                  name=f"sd_{b}_{tch}")
                    nc.scalar.activation(out=sd[:], in_=mvt[:, :, 1], func=AF.Sqrt,
                                         bias=eps_sb[:])
                    rs = lnA.tile([128, 4], f32, tag="rs", bufs=2,
                                  name=f"rs_{b}_{tch}")
                    nc.vector.reciprocal(out=rs[:], in_=sd[:])
                    for sub in range(4):
                        h = lnA.tile([128, C], bf16, tag="h", bufs=5,
                                     name=f"h_{b}_{tch}_{sub}")
                        nc.vector.tensor_scalar(out=h[:], in0=hsubs[sub][:],
                                                scalar1=mvt[:, sub, 0:1],
                                                scalar2=rs[:, sub:sub + 1],
                                                op0=OP.subtract,
                                                op1=OP.mult)
                        hsubs[sub] = h
                    # transpose h (bf16) -> hTb [c, t]; LN1 affine in copy
                    hTb = lnA.tile([128, 8, 512], bf16, tag="hTb", bufs=2,
                                   name=f"hTb_{b}_{tch}")
                    for cc in range(8):
                        pth = psA0.tile([128, 512], bf16, tag="pth", bufs=2,
                                        name=f"pth_{b}_{tch}_{cc}")
                        for sub in range(4):
                            nc.tensor.transpose(
                                pth[:, sub * 128:(sub + 1) * 128],
                                hsubs[sub][:, cc * 128:(cc + 1) * 128], idb_sb[:])
                        nc.scalar.copy(out=hTb[:, cc, :], in_=pth[:])
                    # q^T, k^T (bf16 matmul -> bf16 store)
                    col = tch * 512
                    for w_sb, dst, brow in ((wq_sb, qT[b], bq_sb),
                                            (wk_sb, kT[b], bk_sb)):
                        pqk = psA0.tile([128, 512], f32, tag="pqv", bufs=2,
                                        name=f"pqk_{b}_{tch}_{dst.name}")
                        nc.tensor.matmul(pqk[:], brow[:], ones512_sb[:],
                                         start=True, stop=False)
                        for cc in range(8):
                            nc.tensor.matmul(pqk[:], w_sb[:, cc, :],
                                             hTb[:, cc, :],
                                             start=False, stop=(cc == 7))
                        nc.scalar.copy(out=dst[:, col:col + 512], in_=pqk[:])
                    # v (+ ones column), bf16
                    for sub in range(4):
                        sb = tch * 4 + sub
                        pv = psA0.tile([128, 512], f32, tag="pqv", bufs=2,
                                       name=f"pv_{b}_{sb}")
                        for cc in range(8):
                            nc.tensor.matmul(
                                pv[:, 0:130], hTb[:, cc, sub * 128:(sub + 1) * 128],
                                wv_sb[:, cc, :], start=(cc == 0), stop=False)
                        nc.tensor.matmul(pv[:, 0:130], ones_b_sb[:], onespat_sb[:],
                                         start=False, stop=True)
                        nc.scalar.copy(out=vaug[b][:, sb * 130:(sb + 1) * 130],
                                       in_=pv[:, 0:130])

    # ======================================================================
    # Phase B: causal attention, query-half-major; per-half A2A
    # ======================================================================
    nc.sync.dma_start(out=wp_sb[:],
                      in_=wproj[:].rearrange("(dc p) e -> p dc e", p=128))
    nc.sync.dma_start(out=xo[:], in_=x_own[:].rearrange("(tq p) e -> p tq e", p=128))

    persD = tc.alloc_tile_pool(name="persD", bufs=1)
    x2 = persD.tile([128, 4, C], f32, name="x2")
    h2T = persD.tile([128, 8, 512], bf16, name="h2T")
    ff1T = persD.tile([128, 32, 512], bf16, name="ff1T")
    aT_own = persD.tile([128, 8, 512], bf16, name="aT_own")

    with tc.tile_pool(name="attnp", bufs=1) as atp, \
         tc.tile_pool(name="psAB", bufs=1, space="PSUM") as psA:
        dens = [atp.tile([65, 2, T], bf16, tag="den", bufs=2, name=f"den_{b}")
                for b in range(2)]
        for half in range(2):
            qc0 = half * 1024
            nsb = 8 * half + 8
            for h in range(2):
                hp = 64 * h
                for b in range(2):
                    den = dens[b]
                    with nc.named_scope(f"attn_b{b}h{h}q{half}"):
                        pat = [psA.tile([65, 512], f32, tag="pat", bufs=2,
                                        name=f"pat_{b}_{h}_{half}_{i}")
                               for i in range(2)]
                        for sb in range(nsb):
                            # active query chunks of this half (causal)
                            act0 = 0 if sb < 8 * half + 4 else 1
                            dtc = sb // 4 - 2 * half  # diag chunk idx in half
                            ps = psA.tile([128, 1024], f32, tag="ps", bufs=2,
                                          name=f"ps_{b}_{h}_{half}_{sb}")
                            for i in range(act0, 2):
                                nc.tensor.matmul(
                                    ps[:, i * 512:(i + 1) * 512],
                                    kT[b][hp:hp + 64, sb * 128:sb * 128 + 128],
                                    qT[b][hp:hp + 64,
                                          qc0 + i * 512:qc0 + (i + 1) * 512],
                                    start=True, stop=True)
                            pt = atp.tile([128, 1024], bf16, tag="pt", bufs=4,
                                          name=f"pt_{b}_{h}_{half}_{sb}")
                            nc.scalar.activation(out=pt[:, act0 * 512:1024],
                                                 in_=ps[:, act0 * 512:1024],
                                                 func=AF.Exp, scale=0.125)
                            if dtc >= act0:
                                nc.vector.tensor_mul(
                                    pt[:, dtc * 512:(dtc + 1) * 512],
                                    pt[:, dtc * 512:(dtc + 1) * 512],
                                    mask_sb[:, sb % 4, :])
                            vs = sb * 130 + 65 * h
                            for i in range(act0, 2):
                                last = 8 * half + 3 if i == 0 else nsb - 1
                                nc.tensor.matmul(
                                    pat[i][:], vaug[b][:, vs:vs + 65],
                                    pt[:, i * 512:(i + 1) * 512],
                                    start=(sb == 0), stop=(sb == last))
                        for i in range(2):
                            qcol = qc0 + i * 512
                            nc.vector.tensor_copy(
                                out=aT_h[b][h][:, qcol:qcol + 512],
                                in_=pat[i][0:64, :])
                            nc.vector.tensor_copy(
                                out=den[64:65, h, qcol:qcol + 512],
                                in_=pat[i][64:65, :])
            # shard DMAs + collective for this query half (both heads +
            # denominators ride together)
            for j in range(8):
                bj, tq = j // 4, j % 4
                t0 = qc0 + tq * 256
                nc.sync.dma_start(out=a2a_in[half][j, 0:64, :],
                                  in_=aT_h[bj][0][:, t0:t0 + 256])
                nc.sync.dma_start(out=a2a_in[half][j, 64:128, :],
                                  in_=aT_h[bj][1][:, t0:t0 + 256])
                nc.sync.dma_start(out=a2a_in[half][j, 128:129, :],
                                  in_=dens[bj][64:65, 0, t0:t0 + 256])
                nc.sync.dma_start(out=a2a_in[half][j, 129:130, :],
                                  in_=dens[bj][64:65, 1, t0:t0 + 256])
            nc.gpsimd.collective_compute(
                "AllToAll", mybir.AluOpType.bypass,
                replica_groups=[list(range(NCORES))],
                ins=[a2a_in[half][:].opt()], outs=[a2a_out[half][:].opt()])

    # ----------------------------------------------------------------------
    # Post-A2A per half: normalize + proj + residual + LN2 + FFN1(half)
    # (the half-0 instance of this work overlaps the half-1 A2A)
    # ----------------------------------------------------------------------
    with tc.tile_pool(name="prDE", bufs=1) as prD:
        with tc.tile_pool(name="psD", bufs=1, space="PSUM") as psD:
            for half in range(2):
                hcol = half * 256
                for r in range(8):
                    nc.sync.dma_start(out=aT_own[0:64, r, hcol:hcol + 256],
                                      in_=a2a_out[half][r, 0:64, :])
                    nc.sync.dma_start(out=aT_own[64:128, r, hcol:hcol + 256],
                                      in_=a2a_out[half][r, 64:128, :])
                # softmax normalization: one fast reciprocal for all 16 rows
                dgb = prD.tile([16, 256], bf16, tag="dgb", bufs=2,
                               name=f"dgb_{half}")
                for hh in range(2):
                    for r in range(8):
                        nc.sync.dma_start(
                            out=dgb[hh * 8 + r:hh * 8 + r + 1, :],
                            in_=a2a_out[half][r, 128 + hh:129 + hh, :])
                dg = prD.tile([16, 256], f32, tag="dg", bufs=2,
                              name=f"dg_{half}")
                nc.vector.tensor_copy(out=dg[:], in_=dgb[:])
                drc = prD.tile([16, 256], f32, tag="drc", bufs=2,
                               name=f"drc_{half}")
                nc.vector.reciprocal_approx_fast(out=drc[:], in_=dg[:])
                drb = prD.tile([16, 256], bf16, tag="drb", bufs=2,
                               name=f"drb_{half}")
                nc.vector.tensor_copy(out=drb[:], in_=drc[:])
                nc.sync.dma_start(out=dr_dram[half][:], in_=drb[:])
                for r in range(8):
                    rb = prD.tile([128, 256], bf16, tag="rb", bufs=3,
                                  name=f"rb_{half}_{r}")
                    nc.sync.dma_start(
                        out=rb[0:64, :],
                        in_=dr_dram[half][r:r + 1, :].to_broadcast([64, 256]))
                    nc.sync.dma_start(
                        out=rb[64:128, :],
                        in_=dr_dram[half][8 + r:9 + r, :].to_broadcast([64, 256]))
                    nc.vector.tensor_mul(aT_own[:, r, hcol:hcol + 256],
                                         aT_own[:, r, hcol:hcol + 256], rb[:])
                # proj + residual + LN2 stats for this half's 2 t-blocks
                mv2 = prD.tile([128, 2, 2], f32, tag="mv2", bufs=2,
                               name=f"mv2_{half}")
                with nc.named_scope(f"proj_ln2_q{half}"):
                    for blk2 in range(2):
                        tq = half * 2 + blk2
                        for eh in range(2):
                            pp = psD.tile([128, 512], f32, tag="pp", bufs=2,
                                          name=f"pp_{tq}_{eh}")
                            nc.tensor.matmul(pp[:], ones_b_sb[:],
                                             bproj_sb[0:1, eh * 512:eh * 512 + 512],
                                             start=True, stop=False)
                            for dc in range(8):
                                nc.tensor.matmul(
                                    pp[:], aT_own[:, dc, tq * 128:(tq + 1) * 128],
                                    wp_sb[:, dc, eh * 512:eh * 512 + 512],
                                    start=False, stop=(dc == 7))
                            nc.vector.tensor_add(
                                x2[:, tq, eh * 512:eh * 512 + 512],
                                pp[:], xo[:, tq, eh * 512:eh * 512 + 512])
                        st2 = prD.tile([128, 2, 6], f32, tag="st2", bufs=2,
                                       name=f"st2_{tq}")
                        nc.vector.bn_stats(out=st2[:, 0, :], in_=x2[:, tq, 0:512])
                        nc.vector.bn_stats(out=st2[:, 1, :], in_=x2[:, tq, 512:1024])
                        nc.vector.bn_aggr(out=mv2[:, blk2, :], in_=st2[:])
                    sd2 = prD.tile([128, 2], f32, tag="sd2", bufs=2,
                                   name=f"sd2_{half}")
                    nc.scalar.activation(out=sd2[:], in_=mv2[:, :, 1],
                                         func=AF.Sqrt, bias=eps_sb[:])
                    rs2 = prD.tile([128, 2], f32, tag="rs2", bufs=2,
                                   name=f"rs2_{half}")
                    nc.vector.reciprocal(out=rs2[:], in_=sd2[:])
                    h2subs = []
                    for blk2 in range(2):
                        tq = half * 2 + blk2
                        h2 = prD.tile([128, C], f32, tag="h2", bufs=3,
                                      name=f"h2_{tq}")
                        nc.vector.tensor_scalar(out=h2[:], in0=x2[:, tq, :],
                                                scalar1=mv2[:, blk2, 0:1],
                                                scalar2=rs2[:, blk2:blk2 + 1],
                                                op0=OP.subtract, op1=OP.mult)
                        h2subs.append(h2)
                    for cc in range(8):
                        pt2 = psD.tile([128, 256], f32, tag="pt2", bufs=2,
                                       name=f"pt2_{half}_{cc}")
                        for blk2 in range(2):
                            nc.tensor.transpose(
                                pt2[:, blk2 * 128:(blk2 + 1) * 128],
                                h2subs[blk2][:, cc * 128:(cc + 1) * 128],
                                id_sb[:])
                        nc.scalar.copy(out=h2T[:, cc, hcol:hcol + 256],
                                       in_=pt2[:])
                # FFN1 for this half's 256 tokens (N=256 matmuls; the half-0
                # instance overlaps the half-1 A2A)
                with nc.named_scope(f"ffn1_q{half}"):
                    for w in range(16):  # m-windows of 256
                        w1w = prD.tile([128, 8, 256], bf16, tag="w1w", bufs=3,
                                       name=f"w1w_{half}_{w}")
                        nc.sync.dma_start(out=w1w[:], in_=w1blk[w])
                        for m2 in range(2):
                            m = w * 2 + m2  # m-chunk of 128
                            pf = psD.tile([128, 256], f32, tag="pf", bufs=3,
                                          name=f"pf_{half}_{m}")
                            for cc in range(8):
                                nc.tensor.matmul(
                                    pf[:], w1w[:, cc, m2 * 128:(m2 + 1) * 128],
                                    h2T[:, cc, hcol:hcol + 256],
                                    start=(cc == 0), stop=(cc == 7))
                            nc.scalar.activation(out=ff1T[:, m, hcol:hcol + 256],
                                                 in_=pf[:], func=AF.Relu,
                                                 bias=b1_sb[:, m:m + 1])
            # --------------------------------------------------------------
            # FFN2 over all 4 t-blocks + residual + store
            # --------------------------------------------------------------
            with tc.tile_pool(name="ps2", bufs=1, space="PSUM") as ps2p, \
                 nc.named_scope("ffn2"):
                pso = [ps2p.tile([128, C], f32, tag="pso", bufs=4,
                                 name=f"pso_{tq}") for tq in range(4)]
                for mc in range(32):
                    w2t = prD.tile([128, C], bf16, tag="w2t", bufs=6,
                                   name=f"w2t_{mc}")
                    nc.sync.dma_start(out=w2t[:], in_=w2[mc * 128:(mc + 1) * 128, :])
                    for tq in range(4):
                        for eh in range(2):
                            nc.tensor.matmul(
                                pso[tq][:, eh * 512:(eh + 1) * 512],
                                ff1T[:, mc, tq * 128:(tq + 1) * 128],
                                w2t[:, eh * 512:(eh + 1) * 512],
                                start=False, stop=(mc == 31))
                for tq in range(4):
                    for eh in range(2):
                        nc.tensor.matmul(pso[tq][:, eh * 512:(eh + 1) * 512],
                                         ones_b_sb[:],
                                         b2r_sb[0:1, eh * 512:(eh + 1) * 512],
                                         start=False, stop=True)
                    ot = prD.tile([128, C], f32, tag="ot", bufs=2,
                                  name=f"ot_{tq}")
                    nc.vector.tensor_add(ot[:], pso[tq][:], x2[:, tq, :])
                    nc.sync.dma_start(out=out[tq * 128:(tq + 1) * 128, :], in_=ot[:])
    persD.release()
    persA.release()
    consts.release()
    dram.release()


# --------------------------------------------------------------------------
# host driver
# --------------------------------------------------------------------------
def _make_in_maps(inputs):
    x = np.ascontiguousarray(np.asarray(inputs["x"], np.float32))
    wq = np.asarray(inputs["wq"], np.float32)
    wk = np.asarray(inputs["wk"], np.float32)
    wv = np.asarray(inputs["wv"], np.float32)
    w_proj = np.ascontiguousarray(np.asarray(inputs["w_proj"], np.float32))
    b_proj = np.asarray(inputs["b_proj"], np.float32)
    w1 = np.ascontiguousarray(np.asarray(inputs["w1"], np.float32))
    b1 = np.asarray(inputs["b1"], np.float32)
    w2 = np.ascontiguousarray(np.asarray(inputs["w2"], np.float32))
    b2 = np.asarray(inputs["b2"], np.float32)
    g1 = np.asarray(inputs["g1"], np.float32)
    be1 = np.asarray(inputs["be1"], np.float32)
    g2 = np.asarray(inputs["g2"], np.float32)
    be2 = np.asarray(inputs["be2"], np.float32)

    xf = x.reshape(BT, C)
    i_mask = np.zeros((4, 128, 512), np.float32)
    s_idx = np.arange(128)[:, None]
    t_idx = np.arange(512)[None, :]
    for i in range(4):
        i_mask[i] = (s_idx + 128 * i <= t_idx).astype(np.float32)
    # LN affine folding: h*g + be followed by W  ==  h @ (g*W) + be@W.
    # gamma scales weight rows; beta becomes a constant bias row.
    w1f = g2[:, None] * w1
    b1f = b1 + be2 @ w1

    common = dict(
        x_bf=xf.astype(ml_dtypes.bfloat16),
        masks=i_mask.astype(ml_dtypes.bfloat16),
        ones_b=np.ones((1, 128), ml_dtypes.bfloat16),
        ones512=np.ones((1, 512), ml_dtypes.bfloat16),
        wproj=w_proj.astype(ml_dtypes.bfloat16),
        bproj=np.ascontiguousarray(b_proj[None, :]).astype(ml_dtypes.bfloat16),
        w1blk=np.ascontiguousarray(
            w1f.reshape(8, 128, 16, 256).transpose(2, 1, 0, 3)).astype(
                ml_dtypes.bfloat16),
        w2=w2.astype(ml_dtypes.bfloat16),
        b1t=np.ascontiguousarray(b1f.reshape(FF // 128, 128).T),
        b2row=np.ascontiguousarray(b2[None, :]).astype(ml_dtypes.bfloat16),

        ident=np.eye(128, dtype=np.float32),
        identb=np.eye(128).astype(ml_dtypes.bfloat16),
    )
    in_maps = []
    for c in range(NCORES):
        b, q = c // 4, c % 4
        t0 = q * 256
        wva = np.zeros((C, 130), np.float32)
        wva[:, 0:64] = wv[2 * c]
        wva[:, 65:129] = wv[2 * c + 1]
        wq2c = np.concatenate([wq[2 * c], wq[2 * c + 1]], axis=1)
        wk2c = np.concatenate([wk[2 * c], wk[2 * c + 1]], axis=1)
        onespat = np.zeros((1, 130), np.float32)
        onespat[0, 64] = 1.0
        onespat[0, 129] = 1.0
        onespat[0] += be1 @ wva  # LN1 beta folded into the v-bias row
        m = dict(common)
        m["x_own"] = np.ascontiguousarray(np.concatenate(
            [xf[b * T + t0: b * T + t0 + 256],
             xf[b * T + 1024 + t0: b * T + 1024 + t0 + 256]], axis=0))
        m["wq2"] = np.ascontiguousarray(g1[:, None] * wq2c).astype(
            ml_dtypes.bfloat16)
        m["wk2"] = np.ascontiguousarray(g1[:, None] * wk2c).astype(
            ml_dtypes.bfloat16)
        m["wv_aug"] = (g1[:, None] * wva).astype(ml_dtypes.bfloat16)
        m["bq_row"] = np.ascontiguousarray(
            (be1 @ wq2c)[None, :]).astype(ml_dtypes.bfloat16)
        m["bk_row"] = np.ascontiguousarray(
            (be1 @ wk2c)[None, :]).astype(ml_dtypes.bfloat16)
        m["onespat"] = onespat.astype(ml_dtypes.bfloat16)
        in_maps.append(m)
    return in_maps


LAST_RESULTS = None


def kernel(trace=False, **inputs):
    global LAST_RESULTS
    from concourse import bass_utils

    if "nc" not in _CACHE:
        _CACHE["nc"] = _build_program()
    nc = _CACHE["nc"]
    in_maps = _make_in_maps(inputs)
    res = bass_utils.run_bass_kernel_spmd(
        nc, in_maps, core_ids=list(range(NCORES)), trace=trace)
    LAST_RESULTS = res
    out = np.zeros((B, T, C), np.float32)
    for c in range(NCORES):
        b, q = c // 4, c % 4
        t0 = q * 256
        r = res.results[c]["out"]
        out[b, t0:t0 + 256, :] = r[0:256]
        out[b, 1024 + t0:1024 + t0 + 256, :] = r[256:512]
    return out
